# revision 1
# baseline (speedup 1.0000x reference)
# Trainium2 Bass kernel for the FGAT message-passing model.
#
# Strategy (8 NeuronCores, SPMD):
#   * Host sorts edges by dst and shards them by destination-node range:
#     each core owns 49 windows of 128 consecutive nodes and all edges
#     pointing into them (padded to 34 groups of 128 edges per window).
#   * Node features h live in an SBUF-resident token table (bf16, 256B
#     tokens). dma_gather(transpose=True) produces feature-major h[src]
#     tiles that are used directly as matmul lhsT operands. int16 gather
#     indices cover only 32768 tokens, so the table is split into lo/hi
#     halves with zero-rows; each edge gathers from both halves (the miss
#     side returns zeros) and the two matmuls accumulate in PSUM.
#   * Edge softmax needs no max subtraction (logits are O(1)); the
#     division by the softmax denominator is applied per *node* after
#     aggregation: agg[n] = (sum_e z_e * hh[src_e]) / (sum_e z_e).
#   * Segment sums are one-hot matmuls: onehot[e, n] = (dstlocal[e] == n)
#     built by a DVE iota compare, accumulated into a per-window PSUM tile.
#   * Between layers the per-core node slices are AllGathered (DRAM).
#   * The 4096-pair MLP readout is sharded 512 pairs/core and accumulated
#     in PSUM across layers.

import numpy as np
import ml_dtypes

import concourse.bacc as bacc
import concourse.bass as bass
import concourse.mybir as mybir
import concourse.tile as tile
from concourse import bass_utils

BF16 = mybir.dt.bfloat16
F32 = mybir.dt.float32
I16 = mybir.dt.int16
AF = mybir.ActivationFunctionType
OP = mybir.AluOpType

# ---------------- problem config ----------------
N_NODES = 50000
N_EDGES = 1600000
N_PAIRS = 4096
IN_N, IN_E = 32, 8
H, OE, ON = 4, 8, 32
L = 4
NC = 8

NW = 49          # windows per core (128 dst nodes each)
G = 34           # edge groups (x128) per window
WIN_E = G * 128  # 4352 edge slots per window
OWN = NW * 128   # 6272 nodes per core
NP = NC * OWN    # 50176 padded node rows
PAIRS_PC = N_PAIRS // NC  # 512

# token-table geometry (gather source). Node chunks of 1024 rows staged as
# (128 partitions x 8 ranks); token(n) = (rank0(chunk)+n%8)*128 + (n%1024)//8.
LO_CHUNKS = 31              # chunks 0..30 -> ranks 1..248 (tokens < 32768)
HI_RANK0 = 257              # chunks 31..48 -> ranks 257..400
N_CHUNKS = NP // 1024       # 49
TAB_RANKS = HI_RANK0 + (N_CHUNKS - LO_CHUNKS) * 8  # 401
OWN_RANKS = 1 + (OWN + 127) // 128 + 7             # zero rank + 49 ranks + chunk slack
TOK = 128                   # bf16 elems per token (256B)

CG_SPLITS = [4, 4, 4, 4, 4, 4, 4, 4, 2]  # chunking of the 34 groups
DEBUG_L0 = False
assert sum(CG_SPLITS) == G


def _token_of(n):
    """Global-table token id for node row n (vectorized)."""
    n = np.asarray(n, np.int64)
    k = n // 1024
    rank0 = np.where(k < LO_CHUNKS, 1 + 8 * k, HI_RANK0 + 8 * (k - LO_CHUNKS))
    return (rank0 + n % 8) * 128 + (n % 1024) // 8


def _token_own(m):
    m = np.asarray(m, np.int64)
    return (1 + 8 * (m // 1024) + m % 8) * 128 + (m % 1024) // 8


def _wrap_idx(idx):
    """dma_gather int16 index layout: (16, n/16) wrapped, replicated to 128."""
    idx = np.asarray(idx, np.int16)
    n = idx.shape[-1]
    w = idx.reshape(*idx.shape[:-1], n // 16, 16)
    w = np.swapaxes(w, -1, -2)  # (.., 16, n/16)
    reps = [1] * (w.ndim - 2) + [8, 1]
    return np.tile(w, reps)  # (.., 128, n/16)


def _prep_inputs(x, e, W_ni, W_nj, W_fij, b_edge, attn, W_node, W1, b1, W2, b2,
                 src, dst, users, items):
    """Build per-core input dicts (host-side sharding; index math only)."""
    bf = ml_dtypes.bfloat16
    x_full = np.zeros((NP, IN_N), np.float32)
    x_full[:N_NODES] = x

    order = np.argsort(dst, kind="stable")
    s_src = src[order].astype(np.int64)
    s_dst = dst[order].astype(np.int64)
    s_eid = order
    # window boundaries: global window w covers nodes [w*128, w*128+128)
    wfirst = np.searchsorted(s_dst, np.arange(0, NP + 128, 128))

    tok_src = _token_of(s_src)

    wcat = np.concatenate([W_ni, W_node], axis=2).astype(bf)          # (L,32,160)
    wnj = W_nj.astype(bf)                                             # (L,32,32)
    wfe = np.concatenate([W_fij, b_edge[:, None, :]], axis=1).astype(bf)  # (L,9,32)
    attnrep = np.tile(attn.reshape(L, 1, H * OE), (1, 128, 1)).astype(bf)  # (L,128,32)
    w1b = np.stack([W1[:L * ON].reshape(L, ON, 128),
                    W1[L * ON:].reshape(L, ON, 128)], axis=1).astype(bf)  # (L,2,32,128)
    b1r = b1.reshape(1, 128).astype(bf)
    w2rep = np.tile(W2.reshape(1, 128), (128, 1)).astype(np.float32)
    b2s = float(b2[0])

    shared = dict(x_full=x_full, wcat=np.ascontiguousarray(wcat),
                  wnj=np.ascontiguousarray(wnj), wfe=np.ascontiguousarray(wfe),
                  attnrep=np.ascontiguousarray(attnrep),
                  w1b=np.ascontiguousarray(w1b), b1r=b1r, w2rep=w2rep)

    eT_all = np.zeros((IN_E + 1, N_EDGES), bf)
    eT_all[:IN_E] = e.T.astype(bf)
    eT_all[IN_E] = 1.0

    in_maps = []
    for c in range(NC):
        base = c * OWN
        idx_lo = np.zeros((NW, WIN_E), np.int16)
        idx_hi = np.zeros((NW, WIN_E), np.int16)
        idx_dst = np.zeros((NW, WIN_E), np.int16)
        dstloc = np.full((NW, 128, G), 200.0, np.float32)
        eTw = np.zeros((NW, IN_E + 1, WIN_E), bf)
        for w in range(NW):
            gw = c * NW + w
            if gw >= NP // 128:
                continue
            lo, hi = wfirst[gw], wfirst[gw + 1]
            ne = hi - lo
            if ne == 0:
                continue
            assert ne <= WIN_E
            tt = tok_src[lo:hi]
            idx_lo[w, :ne] = np.where(tt < 32768, tt, 0).astype(np.int16)
            idx_hi[w, :ne] = np.where(tt >= 32768, tt - 32768, 0).astype(np.int16)
            idx_dst[w, :ne] = _token_own(s_dst[lo:hi] - base).astype(np.int16)
            sl = np.arange(lo, hi)
            p = (sl - lo) % 128
            g = (sl - lo) // 128
            dstloc[w, p, g] = (s_dst[lo:hi] - base - w * 128).astype(np.float32)
            eTw[w, :, :ne] = eT_all[:, s_eid[lo:hi]]
        u = users[c * PAIRS_PC:(c + 1) * PAIRS_PC].astype(np.int64)
        it = items[c * PAIRS_PC:(c + 1) * PAIRS_PC].astype(np.int64)
        tu, ti = _token_of(u), _token_of(it)
        m = dict(shared)
        m.update(
            x_own=np.ascontiguousarray(x_full[base:base + OWN]),
            idx_lo=_wrap_idx(idx_lo), idx_hi=_wrap_idx(idx_hi),
            idx_dst=_wrap_idx(idx_dst),
            dstloc=dstloc, eTw=np.ascontiguousarray(eTw),
            u_lo=_wrap_idx(np.where(tu < 32768, tu, 0)),
            u_hi=_wrap_idx(np.where(tu >= 32768, tu - 32768, 0)),
            i_lo=_wrap_idx(np.where(ti < 32768, ti, 0)),
            i_hi=_wrap_idx(np.where(ti >= 32768, ti - 32768, 0)),
        )
        in_maps.append(m)
    return in_maps, b2s


def _build(nc, b2s):
    """Emit the SPMD kernel (identical program for all 8 cores)."""
    t_in = {}
    for name, shape, dt in [
        ("x_full", (NP, IN_N), F32), ("x_own", (OWN, IN_N), F32),
        ("wcat", (L, 32, 160), BF16), ("wnj", (L, 32, 32), BF16),
        ("wfe", (L, 9, 32), BF16), ("attnrep", (L, 128, 32), BF16),
        ("w1b", (L, 2, 32, 128), BF16), ("b1r", (1, 128), BF16),
        ("w2rep", (128, 128), F32),
        ("idx_lo", (NW, 128, WIN_E // 16), I16),
        ("idx_hi", (NW, 128, WIN_E // 16), I16),
        ("idx_dst", (NW, 128, WIN_E // 16), I16),
        ("dstloc", (NW, 128, G), F32),
        ("eTw", (NW, IN_E + 1, WIN_E), BF16),
        ("u_lo", (128, PAIRS_PC // 16), I16), ("u_hi", (128, PAIRS_PC // 16), I16),
        ("i_lo", (128, PAIRS_PC // 16), I16), ("i_hi", (128, PAIRS_PC // 16), I16),
    ]:
        t_in[name] = nc.dram_tensor(name, list(shape), dt, kind="ExternalInput").ap()
    outp = nc.dram_tensor("outp", [PAIRS_PC, 1], F32, kind="ExternalOutput").ap()
    global DEBUG_L0
    if DEBUG_L0:
        t_in["hdbg"] = nc.dram_tensor("hdbg", [L * OWN, IN_N], F32,
                                      kind="ExternalOutput").ap()
        t_in["rodbg"] = nc.dram_tensor("rodbg", [PAIRS_PC, 128], F32,
                                       kind="ExternalOutput").ap()

    with tile.TileContext(nc) as tc:
        _emit(nc, tc, t_in, outp, b2s)
    return t_in, outp


def _emit(nc, tc, t, outp, b2s):
    import contextlib
    ctx = contextlib.ExitStack()
    with ctx:
        persist = ctx.enter_context(tc.tile_pool(name="persist", bufs=1))
        dram = ctx.enter_context(tc.tile_pool(name="dram", bufs=1, space="DRAM"))
        winp = ctx.enter_context(tc.tile_pool(name="winp", bufs=2))
        chkp = ctx.enter_context(tc.tile_pool(name="chkp", bufs=3))
        pep = ctx.enter_context(tc.tile_pool(name="pep", bufs=2, space="PSUM"))
        pwin = ctx.enter_context(tc.tile_pool(name="pwin", bufs=2, space="PSUM"))
        pro = ctx.enter_context(tc.tile_pool(name="pro", bufs=1, space="PSUM"))
        stgp = ctx.enter_context(tc.tile_pool(name="stgp", bufs=2))
        flup = ctx.enter_context(tc.tile_pool(name="flup", bufs=2))

        # ---- persistent tiles ----
        tab = persist.tile([128, TAB_RANKS * TOK], BF16, tag="tab")
        tab_own = persist.tile([128, OWN_RANKS * TOK], BF16, tag="tabown")
        iota_i = persist.tile([128, 128], mybir.dt.int32, tag="iotai")
        iota_f = persist.tile([128, 128], F32, tag="iotaf")
        ones1 = persist.tile([1, 128], BF16, tag="ones1")
        w2r = persist.tile([128, 128], F32, tag="w2r")
        wcat_s = persist.tile([32, L, 160], BF16, tag="wcat")
        wnj_s = persist.tile([32, L, 32], BF16, tag="wnj")
        wfe_s = persist.tile([9, L, 32], BF16, tag="wfe")
        attn_s = persist.tile([128, L, 32], BF16, tag="attn")
        w1b_s = persist.tile([32, L, 2, 128], BF16, tag="w1b")
        b1_s = persist.tile([1, 128], BF16, tag="b1")
        z1 = pro.tile([128, 512], F32, tag="z1")  # readout accumulator (4x128 pairs)

        nc.gpsimd.memset(tab[:], 0.0)      # zero rows + junk token columns
        nc.gpsimd.memset(tab_own[:], 0.0)
        nc.gpsimd.iota(iota_i[:], pattern=[[1, 128]], base=0, channel_multiplier=0)
        nc.vector.tensor_copy(out=iota_f[:], in_=iota_i[:])
        nc.gpsimd.memset(ones1[:], 1.0)
        nc.sync.dma_start(out=w2r[:], in_=t["w2rep"][:])
        nc.sync.dma_start(out=wcat_s[:], in_=t["wcat"][:].rearrange("l k n -> k l n"))
        nc.sync.dma_start(out=wnj_s[:], in_=t["wnj"][:].rearrange("l k n -> k l n"))
        nc.sync.dma_start(out=wfe_s[:], in_=t["wfe"][:].rearrange("l k n -> k l n"))
        nc.sync.dma_start(out=attn_s[:], in_=t["attnrep"][:].rearrange("l p n -> p l n"))
        nc.sync.dma_start(out=w1b_s[:], in_=t["w1b"][:].rearrange("l s k n -> k l s n"))
        nc.sync.dma_start(out=b1_s[:], in_=t["b1r"][:])

        h_own_d = [dram.tile([OWN, IN_N], F32, name=f"hown{l}", tag=f"hown{l}")
                   for l in range(L)]
        h_full_d = [dram.tile([NP, IN_N], F32, name=f"hfull{l}", tag=f"hfull{l}")
                    for l in range(L)]

        def build_tab(src_dram):
            """(Re)fill the global token table from a (NP,32) f32 DRAM tensor."""
            for k in range(N_CHUNKS):
                stg = stgp.tile([128, 8, IN_N], F32, tag="stg")
                nc.sync.dma_start(
                    out=stg[:],
                    in_=src_dram[k * 1024:(k + 1) * 1024, :]
                    .rearrange("(p j) c -> p j c", p=128, j=8))
                r0 = 1 + 8 * k if k < LO_CHUNKS else HI_RANK0 + 8 * (k - LO_CHUNKS)
                dst = tab[:, r0 * TOK:(r0 + 8) * TOK]
                dst = dst.rearrange("p (j t) -> p j t", j=8)[:, :, :IN_N]
                nc.vector.tensor_copy(out=dst, in_=stg[:])

        def build_tab_own(src_dram):
            for k in range((OWN + 1023) // 1024):
                rows = min(1024, OWN - k * 1024)
                parts = rows // 8
                stg = stgp.tile([128, 8, IN_N], F32, tag="stgo")
                nc.sync.dma_start(
                    out=stg[:parts],
                    in_=src_dram[k * 1024:k * 1024 + rows, :]
                    .rearrange("(p j) c -> p j c", p=parts, j=8))
                r0 = 1 + 8 * k
                dst = tab_own[:parts, r0 * TOK:(r0 + 8) * TOK]
                dst = dst.rearrange("p (j t) -> p j t", j=8)[:, :, :IN_N]
                nc.vector.tensor_copy(out=dst, in_=stg[:parts])

        def gather(out_ap, src_ap, idx_tile, n_idx):
            nc.gpsimd.dma_gather(
                out_ap, src_ap, idx_tile[:], n_idx, n_idx,
                elem_size=TOK, transpose=True, single_packet=False,
                sbuf_tokens_per_rank=128, sbuf_free_dim_per_rank=256,
                sbuf_free_dim_pad_per_rank=0, sbuf_byte_offset=0)

        def readout(l):
            """Accumulate state-l contribution for the 512 pairs of this core."""
            for side, lo_t, hi_t in (("u", t["u_lo"], t["u_hi"]),
                                     ("i", t["i_lo"], t["i_hi"])):
                ilo = flup.tile([128, PAIRS_PC // 16], I16, tag="rilo")
                ihi = flup.tile([128, PAIRS_PC // 16], I16, tag="rihi")
                nc.sync.dma_start(out=ilo[:], in_=lo_t[:])
                nc.sync.dma_start(out=ihi[:], in_=hi_t[:])
                glo = flup.tile([128, 1, PAIRS_PC], BF16, tag="rglo")
                ghi = flup.tile([128, 1, PAIRS_PC], BF16, tag="rghi")
                gather(glo[:], tab[:], ilo, PAIRS_PC)
                gather(ghi[:], tab[:, 256 * TOK:], ihi, PAIRS_PC)
                si = 0 if side == "u" else 1
                for tt in range(4):
                    for gb, st in ((glo, l == 0 and si == 0 and tt == 0),
                                   (ghi, False)):
                        nc.tensor.matmul(
                            out=z1[:, tt * 128:(tt + 1) * 128],
                            lhsT=gb[0:32, 0, tt * 128:(tt + 1) * 128],
                            rhs=w1b_s[:, l, si],
                            start=st, stop=False, skip_group_check=True)

        # ================= layer loop =================
        for l in range(L):
            if l == 0:
                build_tab(t["x_full"])
                build_tab_own(t["x_own"])
            else:
                build_tab(h_full_d[l - 1][:])
                build_tab_own(h_own_d[l - 1][:])
                readout(l - 1)

            for w in range(NW):
                ilo = winp.tile([128, WIN_E // 16], I16, tag="ilo")
                ihi = winp.tile([128, WIN_E // 16], I16, tag="ihi")
                idt = winp.tile([128, WIN_E // 16], I16, tag="idt")
                dsl = winp.tile([128, G], F32, tag="dsl")
                nc.sync.dma_start(out=ilo[:], in_=t["idx_lo"][w])
                nc.sync.dma_start(out=ihi[:], in_=t["idx_hi"][w])
                nc.sync.dma_start(out=idt[:], in_=t["idx_dst"][w])
                nc.sync.dma_start(out=dsl[:], in_=t["dstloc"][w])
                glo = winp.tile([128, 1, WIN_E], BF16, tag="glo")
                ghi = winp.tile([128, 1, WIN_E], BF16, tag="ghi")
                gdt = winp.tile([128, 1, WIN_E], BF16, tag="gdt")
                gather(glo[:], tab[:], ilo, WIN_E)
                gather(ghi[:], tab[:, 256 * TOK:], ihi, WIN_E)
                gather(gdt[:], tab_own[:], idt, WIN_E)

                pw = pwin.tile([128, 512], F32, tag="pw")
                g0 = 0
                for ci, cg in enumerate(CG_SPLITS):
                    pe = pep.tile([128, 1024], F32, tag="pe")
                    eTt = chkp.tile([IN_E + 1, 512], BF16, tag="eT")
                    nc.sync.dma_start(
                        out=eTt[:, :cg * 128],
                        in_=t["eTw"][w, :, g0 * 128:(g0 + cg) * 128])
                    for gi in range(cg):
                        g = g0 + gi
                        es = slice(g * 128, (g + 1) * 128)
                        o = gi * 256
                        nc.tensor.matmul(out=pe[:, o:o + 160],
                                         lhsT=glo[0:32, 0, es], rhs=wcat_s[:, l],
                                         start=True, stop=False)
                        nc.tensor.matmul(out=pe[:, o:o + 160],
                                         lhsT=ghi[0:32, 0, es], rhs=wcat_s[:, l],
                                         start=False, stop=False)
                        nc.tensor.matmul(out=pe[:, o:o + 32],
                                         lhsT=gdt[0:32, 0, es], rhs=wnj_s[:, l],
                                         start=False, stop=False)
                        nc.tensor.matmul(out=pe[:, o:o + 32],
                                         lhsT=eTt[0:9, gi * 128:(gi + 1) * 128], rhs=wfe_s[:, l],
                                         start=False, stop=True)
                    pe3 = pe[:].rearrange("p (g x) -> p g x", g=4)[:, :cg]
                    # one-hot (bf16) from dstlocal vs iota
                    oh = chkp.tile([128, 4, 128], BF16, tag="oh")
                    nc.vector.tensor_tensor(
                        out=oh[:, :cg],
                        in0=dsl[:, g0:g0 + cg].unsqueeze(2).to_broadcast([128, cg, 128]),
                        in1=iota_f[:].unsqueeze(1).to_broadcast([128, cg, 128]),
                        op=OP.is_equal)
                    # leaky relu = max(x, 0.01x) -> f_out bf16
                    sc = chkp.tile([128, 4, 32], F32, tag="sc")
                    nc.vector.tensor_scalar_mul(out=sc[:, :cg], in0=pe3[:, :, 0:32],
                                                scalar1=0.01)
                    fo = chkp.tile([128, 4, 32], BF16, tag="fo")
                    nc.vector.tensor_tensor(out=fo[:, :cg], in0=pe3[:, :, 0:32],
                                            in1=sc[:, :cg], op=OP.max)
                    # logits = sum_d f_out*attn ; z = exp(logits)
                    lm = chkp.tile([128, 4, 32], F32, tag="lm")
                    nc.vector.tensor_tensor(
                        out=lm[:, :cg], in0=fo[:, :cg],
                        in1=attn_s[:, l].unsqueeze(1).to_broadcast([128, cg, 32]),
                        op=OP.mult)
                    lg = chkp.tile([128, 4, 4], F32, tag="lg")
                    nc.vector.tensor_reduce(
                        out=lg[:, :cg],
                        in_=lm[:, :cg].rearrange("p g (h d) -> p g h d", d=OE),
                        axis=mybir.AxisListType.X, op=OP.add)
                    mz = chkp.tile([128, 4, 132], BF16, tag="mz")
                    nc.scalar.activation(out=mz[:, :cg, 128:132], in_=lg[:, :cg],
                                         func=AF.Exp)
                    # msg = z (head-bcast) * hh
                    nc.vector.tensor_tensor(
                        out=mz[:, :cg, 0:128].rearrange("p g (h j) -> p g h j", j=ON),
                        in0=pe3[:, :, 32:160].rearrange("p g (h j) -> p g h j", j=ON),
                        in1=mz[:, :cg, 128:132].unsqueeze(3).to_broadcast([128, cg, 4, ON]),
                        op=OP.mult)
                    for gi in range(cg):
                        g = g0 + gi
                        nc.tensor.matmul(out=pw[:, 0:132],
                                         lhsT=oh[:, gi], rhs=mz[:, gi],
                                         start=(g == 0), stop=(g == G - 1))
                    g0 += cg

                # ---- window flush: agg = num/den, head-sum, ELU ----
                den = flup.tile([128, 4], F32, tag="den")
                nc.vector.tensor_scalar_add(out=den[:], in0=pw[:, 128:132],
                                            scalar1=1e-30)
                rec = flup.tile([128, 4], F32, tag="rec")
                nc.vector.reciprocal(out=rec[:], in_=den[:])
                ag = flup.tile([128, 4, 32], F32, tag="ag")
                nc.vector.tensor_tensor(
                    out=ag[:],
                    in0=pw[:, 0:128].rearrange("p (h j) -> p h j", j=ON),
                    in1=rec[:].unsqueeze(2).to_broadcast([128, 4, ON]),
                    op=OP.mult)
                s = flup.tile([128, 32], F32, tag="s")
                nc.vector.tensor_reduce(
                    out=s[:], in_=ag[:].rearrange("p h j -> p j h"),
                    axis=mybir.AxisListType.X, op=OP.add)
                r = flup.tile([128, 32], F32, tag="r")
                nc.scalar.activation(out=r[:], in_=s[:], func=AF.Relu)
                mn = flup.tile([128, 32], F32, tag="mn")
                nc.vector.tensor_scalar_min(out=mn[:], in0=s[:], scalar1=0.0)
                em = flup.tile([128, 32], F32, tag="em")
                nc.scalar.activation(out=em[:], in_=mn[:], func=AF.Exp)
                hn = flup.tile([128, 32], F32, tag="hn")
                nc.vector.tensor_tensor(out=hn[:], in0=r[:], in1=em[:], op=OP.add)
                nc.vector.tensor_scalar_add(out=hn[:], in0=hn[:], scalar1=-1.0)
                nc.sync.dma_start(out=h_own_d[l][w * 128:(w + 1) * 128, :], in_=hn[:])
                if DEBUG_L0:
                    nc.sync.dma_start(
                        out=t["hdbg"][l * OWN + w * 128:l * OWN + (w + 1) * 128, :],
                        in_=hn[:])

            nc.gpsimd.collective_compute(
                "AllGather", OP.bypass,
                replica_groups=[list(range(NC))],
                ins=[h_own_d[l].opt()], outs=[h_full_d[l].opt()])

        # ================= readout tail =================
        build_tab(h_full_d[L - 1][:])
        readout(L - 1)
        # + b1 (broadcast add via K=1 matmul), close accumulation
        for tt in range(4):
            nc.tensor.matmul(out=z1[:, tt * 128:(tt + 1) * 128],
                             lhsT=ones1[0:1, 0:128], rhs=b1_s[:],
                             start=False, stop=(tt == 3), skip_group_check=True)
        for tt in range(4):
            zr = flup.tile([128, 128], F32, tag="zr")
            nc.scalar.activation(out=zr[:], in_=z1[:, tt * 128:(tt + 1) * 128],
                                 func=AF.Relu)
            if DEBUG_L0:
                nc.sync.dma_start(out=t["rodbg"][tt * 128:(tt + 1) * 128, :],
                                  in_=zr[:])
            zw = flup.tile([128, 128], F32, tag="zw")
            nc.vector.tensor_tensor(out=zw[:], in0=zr[:], in1=w2r[:], op=OP.mult)
            zs = flup.tile([128, 1], F32, tag="zs")
            nc.vector.tensor_reduce(out=zs[:], in_=zw[:],
                                    axis=mybir.AxisListType.X, op=OP.add)
            nc.vector.tensor_scalar_add(out=zs[:], in0=zs[:], scalar1=b2s)
            ot = flup.tile([128, 1], F32, tag="ot")
            nc.scalar.activation(out=ot[:], in_=zs[:], func=AF.Sigmoid)
            nc.sync.dma_start(out=outp[tt * 128:(tt + 1) * 128, :], in_=ot[:])


_COMPILED = {}


def _get_kernel(b2s):
    key = round(b2s, 10)
    if key not in _COMPILED:
        nc = bacc.Bacc("TRN2", target_bir_lowering=False, debug=False,
                       num_devices=NC)
        _build(nc, b2s)
        nc.compile()
        _COMPILED[key] = nc
    return _COMPILED[key]


def kernel(**inputs):
    inputs = {k: np.asarray(v) for k, v in inputs.items()}
    in_maps, b2s = _prep_inputs(**inputs)
    nc = _get_kernel(b2s)
    res = bass_utils.run_bass_kernel_spmd(nc, in_maps, core_ids=list(range(NC)))
    out = np.concatenate([r["outp"][:, 0] for r in res.results])
    return out.astype(np.float32)


if __name__ == "__main__":
    d = np.load("/root/problem/work/inputs.npz")
    ins = {k: d[k] for k in d.files}
    out = kernel(**ins)
    exp = np.load("/root/problem/work/expected.npy")
    rel = np.abs(out - exp) / np.maximum(np.abs(exp), 1e-6)
    print("Relative error:", rel.max())

